# revision 10
# baseline (speedup 1.0000x reference)
"""Trainium2 Bass kernel for nn_AttentionLayer (dense transformer block).

Per-batch computation (reference):
  Wi = wn(in_proj_v, in_proj_g)          # [E, C], rows unit-normed * g
  h  = (x @ Wi.T + b_i + te) * sqrt(.5)  # [Tq, E]
  scores = h @ K                         # [Tq, Ts]   (K = encoder_keys [E, Ts])
  attn = softmax(scores, -1)             # output #2
  ctx = (attn @ V) * 32                  # V = encoder_values [Ts, E]
  out = (ctx @ Wo.T + b_o + x) * sqrt(.5)

Sharding: data-parallel over batch N=16 across 8 cores (2 batches/core).

Per-core dataflow (everything on-device; fp32r for the precision-critical
h/scores path, bf16 for the attn@V / @Wo.T tail):
  mm1: h^T[e,t] = WiT.T @ x^T   -- lhsT = WiT (one-time PE-transposed,
       weight-norm factors folded per-partition), rhs = x^T (PE-transposed
       tiles), te^T fused as transpose-matmuls accumulated into the same
       PSUM group, bias via per-partition Identity-activation evac with
       scale=sqrt(.5).
  mm2: scores[t,s] = hT.T @ K   -- K natural layout, resident per batch.
  softmax over the free dim: DVE reduce_max(negate) -> ACT Exp(bias=-max,
       accum_out=rowsum) -> reciprocal -> normalize -> DMA attn out.
  mm3: ctx^T[e,t] = V.T @ attn^T -- V natural (bf16), attn^T PE-transposed.
  mm4: out[t,c] = ctxT.T @ WoT + b_o (K=1 matmul) + residual (identity
       matmul on x), evac with scale=sqrt(.5); *32 folded into WoT.

The chunk loop is software-pipelined: produce(ch) = {x/te load, x^T, mm1,
mm2, softmax}; consume(ch) = {attn^T, mm3, mm4, out}. consume(ch) is
emitted after produce(ch+1) so the softmax of chunk ch overlaps the
~18us of PE matmul work in produce(ch+1), keeping the PE gap-free (HAM
stays at K=8/8).
"""
import sys
sys.path.insert(0, "/opt/trn_rl_repo")
import math
import numpy as np

N, TQ, TS, C, E = 16, 1024, 1024, 1024, 1024
NCORES = 8
NB = N // NCORES          # batches per core
P = 128
TCH = 256                 # t-chunk (2 t-tiles)
NCH = TQ // TCH           # chunks per batch
NTT = TCH // P            # t-tiles per chunk
SQ5 = math.sqrt(0.5)
CTX_SCALE = TS * math.sqrt(1.0 / TS)   # 32.0

_CACHED = None


def _build():
    import concourse.mybir as mybir
    from concourse import bacc
    from concourse.tile import TileContext
    from concourse.masks import make_identity
    from contextlib import ExitStack

    f32 = mybir.dt.float32
    f32r = mybir.dt.float32r
    bf16 = mybir.dt.bfloat16
    AX = mybir.AxisListType
    ALU = mybir.AluOpType
    ACT_F = mybir.ActivationFunctionType

    nc = bacc.Bacc(trn_type="TRN2")
    X = nc.dram_tensor("x", [NB, TQ, C], f32, kind="ExternalInput")
    TE = nc.dram_tensor("te", [NB, TQ, E], f32, kind="ExternalInput")
    EK = nc.dram_tensor("ek", [NB, E, TS], f32, kind="ExternalInput")
    EV = nc.dram_tensor("ev", [NB, TS, E], f32, kind="ExternalInput")
    WIV = nc.dram_tensor("wiv", [E, C], f32, kind="ExternalInput")
    WIG = nc.dram_tensor("wig", [E], f32, kind="ExternalInput")
    WIB = nc.dram_tensor("wib", [E], f32, kind="ExternalInput")
    WOV = nc.dram_tensor("wov", [C, E], f32, kind="ExternalInput")
    WOG = nc.dram_tensor("wog", [C], f32, kind="ExternalInput")
    WOB = nc.dram_tensor("wob", [C], f32, kind="ExternalInput")
    OUT = nc.dram_tensor("out", [NB, TQ, C], f32, kind="ExternalOutput")
    ATTN = nc.dram_tensor("attn", [NB, TQ, TS], f32, kind="ExternalOutput")

    with TileContext(nc) as tc:
        ctx = ExitStack()
        with ctx:
            wts = ctx.enter_context(tc.tile_pool(name="wts", bufs=1))
            kvp = ctx.enter_context(tc.tile_pool(name="kvp", bufs=1))
            xp = ctx.enter_context(tc.tile_pool(name="xp", bufs=2))
            tep = ctx.enter_context(tc.tile_pool(name="tep", bufs=1))
            xtp = ctx.enter_context(tc.tile_pool(name="xtp", bufs=1))
            htp = ctx.enter_context(tc.tile_pool(name="htp", bufs=1))
            atp = ctx.enter_context(tc.tile_pool(name="atp", bufs=2))
            attp = ctx.enter_context(tc.tile_pool(name="attp", bufs=2))
            ctp = ctx.enter_context(tc.tile_pool(name="ctp", bufs=1))
            op = ctx.enter_context(tc.tile_pool(name="op", bufs=6))
            sqp = ctx.enter_context(tc.tile_pool(name="sqp", bufs=2))
            st = ctx.enter_context(tc.tile_pool(name="st", bufs=16))
            ps = ctx.enter_context(tc.tile_pool(name="ps", bufs=8, space="PSUM"))

            # ---------------- constants ----------------
            id_f = wts.tile([P, P], f32, tag="id_f")
            make_identity(nc, id_f)
            id_r = wts.tile([P, P], f32r, tag="id_r")
            nc.vector.tensor_copy(id_r, id_f.bitcast(f32r))
            ones_r = wts.tile([1, P], f32r, tag="ones")
            ones_f = wts.tile([1, P], f32, tag="ones_f")
            nc.vector.memset(ones_f, 1.0)
            nc.vector.tensor_copy(ones_r, ones_f.bitcast(f32r))

            # in-proj bias: per-partition [e] layout, pre-scaled by sqrt(.5)
            # (small loads go on the gpsimd queue to keep sync free for bulk)
            b_i_s = wts.tile([P, E // P], f32, tag="b_i")
            nc.gpsimd.dma_start(out=b_i_s, in_=WIB.rearrange("(a p) -> p a", p=P))
            nc.vector.tensor_scalar_mul(b_i_s, b_i_s, SQ5)
            # out-proj bias row (raw; sqrt(.5) applied at PSUM evac)
            b_o_row = wts.tile([1, C], f32r, tag="b_o")
            nc.gpsimd.dma_start(out=b_o_row, in_=WOB[None, :].bitcast(f32r))

            # ---------------- weight prep (pipelined over row-tiles) -----
            # wiT[p_c, cc, e] = Wi_eff[e, 128cc+p]   (f32r)  Wi_eff = g/||v||*v
            # woT[p_e, ec, c] = Wo_eff[c, 128ec+p]   (bf16)  Wo_eff = 32*g/||v||*v
            wiT = wts.tile([P, C // P, E], f32r, tag="wiT")
            woT = wts.tile([P, E // P, C], bf16, tag="woT")
            g_i = wts.tile([P, E // P], f32, tag="g_i")
            g_o = wts.tile([P, C // P], f32, tag="g_o")
            nc.gpsimd.dma_start(out=g_i, in_=WIG.rearrange("(a p) -> p a", p=P))
            nc.gpsimd.dma_start(out=g_o, in_=WOG.rearrange("(a p) -> p a", p=P))

            for VSRC, G, WT, scale_extra in (
                (WIV, g_i, wiT, 1.0),
                (WOV, g_o, woT, CTX_SCALE),
            ):
                nrow = VSRC.shape[0] // P
                for rt in range(nrow):  # row-tile of the weight (dim0)
                    vsl = op.tile([P, 1024], f32, tag="stage")
                    nc.sync.dma_start(out=vsl, in_=VSRC[P * rt:P * (rt + 1), :])
                    sq = sqp.tile([P, 1024], f32, tag="sq")
                    ss = st.tile([P, 1], f32, tag="ss")
                    nc.scalar.activation(sq, vsl, ACT_F.Square, accum_out=ss)
                    srt = st.tile([P, 1], f32, tag="ss")
                    nc.scalar.activation(srt, ss, ACT_F.Sqrt)
                    rcs = st.tile([P, 1], f32, tag="ss")
                    nc.vector.reciprocal(rcs, srt)
                    fac = st.tile([P, 1], f32, tag="ss")
                    nc.vector.tensor_tensor(
                        out=fac, in0=rcs, in1=G[:, rt:rt + 1], op=ALU.mult)
                    if scale_extra != 1.0:
                        nc.vector.tensor_scalar_mul(fac, fac, float(scale_extra))
                    nc.vector.tensor_scalar_mul(vsl, vsl, fac)  # in place
                    for ft in range(1024 // P):  # transpose into WT
                        pt = ps.tile([P, P], f32, tag="ps")
                        nc.tensor.transpose(pt, vsl[:, P * ft:P * (ft + 1)], id_f)
                        nc.vector.tensor_copy(
                            WT[:, ft, P * rt:P * (rt + 1)], pt)

            # ---------------- pipelined chunk loop ----------------
            NTOT = NB * NCH
            state = {}

            def produce(ci):
                n, ch = divmod(ci, NCH)
                t0 = ch * TCH
                s = state[ci] = {}
                if ch == 0:
                    K_sb = kvp.tile([P, E // P, TS], f32r, tag="K")
                    nc.sync.dma_start(
                        out=K_sb,
                        in_=EK[n].rearrange("(ec p) s -> p ec s", p=P)
                        .bitcast(f32r))
                    V_bf = kvp.tile([P, TS // P, E], bf16, tag="V")
                    for sc in range(TS // P):
                        vst = op.tile([P, E], f32, tag="stage")
                        nc.sync.dma_start(
                            out=vst, in_=EV[n, P * sc:P * (sc + 1), :])
                        nc.vector.tensor_copy(V_bf[:, sc, :], vst)
                    state[("kv", n)] = (K_sb, V_bf)
                K_sb, V_bf = state[("kv", n)]

                x_t = xp.tile([P, NTT, C], f32r, tag="x")
                nc.sync.dma_start(
                    out=x_t,
                    in_=X[n, t0:t0 + TCH, :]
                    .rearrange("(tt p) c -> p tt c", p=P).bitcast(f32r))
                te_t = tep.tile([P, NTT, E], f32, tag="te")
                nc.sync.dma_start(
                    out=te_t,
                    in_=TE[n, t0:t0 + TCH, :]
                    .rearrange("(tt p) e -> p tt e", p=P))
                s["x"] = x_t

                # x^T tiles: [c-part, t-free]
                xT_t = xtp.tile([P, C // P, TCH], f32r, tag="xT")
                for cc in range(C // P):
                    for tt in range(NTT):
                        pt = ps.tile([P, P], f32r, tag="ps")
                        nc.tensor.matmul(
                            pt, lhsT=x_t[:, tt, P * cc:P * (cc + 1)],
                            rhs=id_r, is_transpose=True,
                            start=True, stop=True)
                        nc.vector.tensor_copy(
                            xT_t[:, cc, P * tt:P * (tt + 1)], pt)

                # mm1: h^T[e,t] (+te^T fused), evac Identity(scale, bias)
                hT_t = htp.tile([P, E // P, TCH], f32r, tag="hT")
                for et in range(E // P):
                    ph = ps.tile([P, TCH], f32, tag="ps")
                    for cc in range(C // P):
                        nc.tensor.matmul(
                            ph, lhsT=wiT[:, cc, P * et:P * (et + 1)],
                            rhs=xT_t[:, cc, :],
                            start=(cc == 0), stop=False)
                    for tt in range(NTT):
                        nc.tensor.matmul(
                            ph[:, P * tt:P * (tt + 1)],
                            lhsT=te_t[:, tt, P * et:P * (et + 1)],
                            rhs=id_f, is_transpose=True,
                            start=False, stop=(tt == NTT - 1))
                    nc.scalar.activation(
                        hT_t[:, et, :], ph, ACT_F.Identity,
                        scale=SQ5, bias=b_i_s[:, et:et + 1])

                # mm2 + softmax per t-tile
                attn_t = atp.tile([P, NTT, TS], f32, tag="attn")
                s["attn"] = attn_t
                for tt in range(NTT):
                    psc = []
                    for sh in range(2):
                        pb = ps.tile([P, 512], f32, tag="ps")
                        for ec in range(E // P):
                            nc.tensor.matmul(
                                pb,
                                lhsT=hT_t[:, ec, P * tt:P * (tt + 1)],
                                rhs=K_sb[:, ec, 512 * sh:512 * (sh + 1)],
                                start=(ec == 0), stop=(ec == 7))
                        psc.append(pb)
                    nm0 = st.tile([P, 1], f32, tag="sm")
                    nm1 = st.tile([P, 1], f32, tag="sm")
                    nc.vector.tensor_reduce(
                        nm0, psc[0], axis=AX.X, op=ALU.max, negate=True)
                    nc.vector.tensor_reduce(
                        nm1, psc[1], axis=AX.X, op=ALU.max, negate=True)
                    nm = st.tile([P, 1], f32, tag="sm")
                    nc.vector.tensor_tensor(
                        out=nm, in0=nm0, in1=nm1, op=ALU.min)
                    sm0 = st.tile([P, 1], f32, tag="sm")
                    sm1 = st.tile([P, 1], f32, tag="sm")
                    for sh, smx in ((0, sm0), (1, sm1)):
                        nc.scalar.activation(
                            attn_t[:, tt, 512 * sh:512 * (sh + 1)], psc[sh],
                            ACT_F.Exp, bias=nm, scale=1.0, accum_out=smx)
                    stot = st.tile([P, 1], f32, tag="sm")
                    nc.vector.tensor_tensor(
                        out=stot, in0=sm0, in1=sm1, op=ALU.add)
                    rcp = st.tile([P, 1], f32, tag="sm")
                    nc.vector.reciprocal(rcp, stot)
                    nc.vector.tensor_scalar_mul(
                        attn_t[:, tt, :], attn_t[:, tt, :], rcp)
                    nc.sync.dma_start(
                        out=ATTN[n, t0 + P * tt:t0 + P * (tt + 1), :],
                        in_=attn_t[:, tt, :])

            def consume(ci):
                n, ch = divmod(ci, NCH)
                t0 = ch * TCH
                s = state[ci]
                K_sb, V_bf = state[("kv", n)]
                attn_t = s["attn"]
                x_t = s["x"]

                attnT_t = attp.tile([P, TS // P, TCH], bf16, tag="attnT")
                for tt in range(NTT):
                    for sc in range(TS // P):
                        pt = ps.tile([P, P], f32, tag="ps")
                        nc.tensor.matmul(
                            pt, lhsT=attn_t[:, tt, P * sc:P * (sc + 1)],
                            rhs=id_f, is_transpose=True,
                            start=True, stop=True)
                        nc.vector.tensor_copy(
                            attnT_t[:, sc, P * tt:P * (tt + 1)], pt)

                # mm3: ctx^T[e,t] = V.T @ attn^T  (bf16)
                ctxT_t = ctp.tile([P, E // P, TCH], bf16, tag="ctxT")
                for et in range(E // P):
                    pc = ps.tile([P, TCH], f32, tag="ps")
                    for sc in range(TS // P):
                        nc.tensor.matmul(
                            pc, lhsT=V_bf[:, sc, P * et:P * (et + 1)],
                            rhs=attnT_t[:, sc, :],
                            start=(sc == 0), stop=(sc == 7))
                    nc.scalar.copy(ctxT_t[:, et, :], pc)

                # mm4: out[t,c] (+bias, +residual), evac * sqrt(.5)
                for tt in range(NTT):
                    out_t = op.tile([P, C], f32, tag="stage")
                    for cf in range(2):
                        po = ps.tile([P, 512], f32, tag="ps")
                        for ec in range(E // P):
                            nc.tensor.matmul(
                                po,
                                lhsT=ctxT_t[:, ec, P * tt:P * (tt + 1)],
                                rhs=woT[:, ec, 512 * cf:512 * (cf + 1)],
                                start=(ec == 0), stop=False)
                        nc.tensor.matmul(
                            po, lhsT=ones_r,
                            rhs=b_o_row[:, 512 * cf:512 * (cf + 1)],
                            start=False, stop=False)
                        nc.tensor.matmul(
                            po, lhsT=id_r,
                            rhs=x_t[:, tt, 512 * cf:512 * (cf + 1)],
                            start=False, stop=True)
                        nc.scalar.activation(
                            out_t[:, 512 * cf:512 * (cf + 1)], po,
                            ACT_F.Copy, scale=SQ5)
                    nc.sync.dma_start(
                        out=OUT[n, t0 + P * tt:t0 + P * (tt + 1), :],
                        in_=out_t)
                del s["attn"], s["x"]

            for ci in range(NTOT + 1):
                if ci < NTOT:
                    produce(ci)
                if ci > 0:
                    consume(ci - 1)

    nc.finalize()
    return nc


def _get_nc():
    global _CACHED
    if _CACHED is None:
        _CACHED = _build()
    return _CACHED


def kernel(x, target_embedding, encoder_keys, encoder_values,
           in_proj_v, in_proj_g, in_proj_b,
           out_proj_v, out_proj_g, out_proj_b,
           trace=False, **run_kwargs):
    from concourse.bass_utils import run_bass_kernel_spmd

    nc = _get_nc()
    f32 = np.float32
    x = np.ascontiguousarray(np.asarray(x, f32))
    te = np.ascontiguousarray(np.asarray(target_embedding, f32))
    ek = np.ascontiguousarray(np.asarray(encoder_keys, f32))
    ev = np.ascontiguousarray(np.asarray(encoder_values, f32))
    wiv = np.ascontiguousarray(np.asarray(in_proj_v, f32))
    wig = np.ascontiguousarray(np.asarray(in_proj_g, f32))
    wib = np.ascontiguousarray(np.asarray(in_proj_b, f32))
    wov = np.ascontiguousarray(np.asarray(out_proj_v, f32))
    wog = np.ascontiguousarray(np.asarray(out_proj_g, f32))
    wob = np.ascontiguousarray(np.asarray(out_proj_b, f32))

    in_maps = []
    for c in range(NCORES):
        sl = slice(NB * c, NB * (c + 1))
        in_maps.append({
            "x": x[sl], "te": te[sl], "ek": ek[sl], "ev": ev[sl],
            "wiv": wiv, "wig": wig, "wib": wib,
            "wov": wov, "wog": wog, "wob": wob,
        })
    res = run_bass_kernel_spmd(nc, in_maps, core_ids=list(range(NCORES)),
                               trace=trace, **run_kwargs)
    out = np.concatenate([r["out"] for r in res.results], axis=0)
    attn = np.concatenate([r["attn"] for r in res.results], axis=0)
    kernel.last_results = res
    return out, attn


# revision 11
# speedup vs baseline: 1.1334x; 1.1334x over previous
"""Trainium2 Bass kernel for nn_AttentionLayer (dense transformer block).

Per-batch computation (reference):
  Wi = wn(in_proj_v, in_proj_g)          # [E, C], rows unit-normed * g
  h  = (x @ Wi.T + b_i + te) * sqrt(.5)  # [Tq, E]
  scores = h @ K                         # [Tq, Ts]   (K = encoder_keys [E, Ts])
  attn = softmax(scores, -1)             # output #2
  ctx = (attn @ V) * 32                  # V = encoder_values [Ts, E]
  out = (ctx @ Wo.T + b_o + x) * sqrt(.5)

Sharding: data-parallel over batch N=16 across 8 cores (2 batches/core).

Per-core dataflow (everything on-device; fp32r for the precision-critical
h/scores path, bf16 for the attn@V / @Wo.T tail):
  mm1: h^T[e,t] = WiT.T @ x^T   -- lhsT = WiT (one-time PE-transposed,
       weight-norm factors folded per-partition), rhs = x^T (PE-transposed
       tiles), te^T fused as transpose-matmuls accumulated into the same
       PSUM group, bias via per-partition Identity-activation evac with
       scale=sqrt(.5).
  mm2: scores[t,s] = hT.T @ K   -- K natural layout, resident per batch.
  softmax over the free dim: DVE reduce_max(negate) -> ACT Exp(bias=-max,
       accum_out=rowsum) -> reciprocal -> normalize -> DMA attn out.
  mm3: ctx^T[e,t] = V.T @ attn^T -- V natural (bf16), attn^T PE-transposed.
  mm4: out[t,c] = ctxT.T @ WoT + b_o (K=1 matmul) + residual (identity
       matmul on x), evac with scale=sqrt(.5); *32 folded into WoT.

The chunk loop is software-pipelined: produce(ch) = {x/te load, x^T, mm1,
mm2, softmax}; consume(ch) = {attn^T, mm3, mm4, out}. consume(ch) is
emitted after produce(ch+1) so the softmax of chunk ch overlaps the
~18us of PE matmul work in produce(ch+1), keeping the PE gap-free (HAM
stays at K=8/8).
"""
import sys
sys.path.insert(0, "/opt/trn_rl_repo")
import math
import numpy as np

N, TQ, TS, C, E = 16, 1024, 1024, 1024, 1024
NCORES = 8
NB = N // NCORES          # batches per core
P = 128
TCH = 256                 # t-chunk (2 t-tiles)
NCH = TQ // TCH           # chunks per batch
NTT = TCH // P            # t-tiles per chunk
SQ5 = math.sqrt(0.5)
CTX_SCALE = TS * math.sqrt(1.0 / TS)   # 32.0

_CACHED = None


def _build():
    import concourse.mybir as mybir
    from concourse import bacc
    from concourse.tile import TileContext
    from concourse.masks import make_identity
    from contextlib import ExitStack

    f32 = mybir.dt.float32
    f32r = mybir.dt.float32r
    bf16 = mybir.dt.bfloat16
    AX = mybir.AxisListType
    ALU = mybir.AluOpType
    ACT_F = mybir.ActivationFunctionType

    nc = bacc.Bacc(trn_type="TRN2")
    X = nc.dram_tensor("x", [NB, TQ, C], f32, kind="ExternalInput")
    TE = nc.dram_tensor("te", [NB, TQ, E], f32, kind="ExternalInput")
    EK = nc.dram_tensor("ek", [NB, E, TS], f32, kind="ExternalInput")
    EV = nc.dram_tensor("ev", [NB, TS, E], f32, kind="ExternalInput")
    WIV = nc.dram_tensor("wiv", [E, C], f32, kind="ExternalInput")
    WIG = nc.dram_tensor("wig", [E], f32, kind="ExternalInput")
    WIB = nc.dram_tensor("wib", [E], f32, kind="ExternalInput")
    WOV = nc.dram_tensor("wov", [C, E], f32, kind="ExternalInput")
    WOG = nc.dram_tensor("wog", [C], f32, kind="ExternalInput")
    WOB = nc.dram_tensor("wob", [C], f32, kind="ExternalInput")
    OUT = nc.dram_tensor("out", [NB, TQ, C], f32, kind="ExternalOutput")
    ATTN = nc.dram_tensor("attn", [NB, TQ, TS], f32, kind="ExternalOutput")

    with TileContext(nc) as tc:
        ctx = ExitStack()
        with ctx:
            wts = ctx.enter_context(tc.tile_pool(name="wts", bufs=1))
            kvp = ctx.enter_context(tc.tile_pool(name="kvp", bufs=1))
            xp = ctx.enter_context(tc.tile_pool(name="xp", bufs=3))
            tep = ctx.enter_context(tc.tile_pool(name="tep", bufs=1))
            xtp = ctx.enter_context(tc.tile_pool(name="xtp", bufs=1))
            htp = ctx.enter_context(tc.tile_pool(name="htp", bufs=1))
            atp = ctx.enter_context(tc.tile_pool(name="atp", bufs=2))
            attp = ctx.enter_context(tc.tile_pool(name="attp", bufs=2))
            ctp = ctx.enter_context(tc.tile_pool(name="ctp", bufs=1))
            op = ctx.enter_context(tc.tile_pool(name="op", bufs=5))
            sqp = ctx.enter_context(tc.tile_pool(name="sqp", bufs=2))
            st = ctx.enter_context(tc.tile_pool(name="st", bufs=16))
            ps = ctx.enter_context(tc.tile_pool(name="ps", bufs=8, space="PSUM"))

            # ---------------- constants ----------------
            id_f = wts.tile([P, P], f32, tag="id_f")
            make_identity(nc, id_f)
            id_r = wts.tile([P, P], f32r, tag="id_r")
            nc.vector.tensor_copy(id_r, id_f.bitcast(f32r))
            ones_r = wts.tile([1, P], f32r, tag="ones")
            ones_f = wts.tile([1, P], f32, tag="ones_f")
            nc.vector.memset(ones_f, 1.0)
            nc.vector.tensor_copy(ones_r, ones_f.bitcast(f32r))

            # in-proj bias: per-partition [e] layout, pre-scaled by sqrt(.5)
            # (small loads go on the gpsimd queue to keep sync free for bulk)
            b_i_s = wts.tile([P, E // P], f32, tag="b_i")
            nc.gpsimd.dma_start(out=b_i_s, in_=WIB.rearrange("(a p) -> p a", p=P))
            nc.vector.tensor_scalar_mul(b_i_s, b_i_s, SQ5)
            # out-proj bias row (raw; sqrt(.5) applied at PSUM evac)
            b_o_row = wts.tile([1, C], f32r, tag="b_o")
            nc.gpsimd.dma_start(out=b_o_row, in_=WOB[None, :].bitcast(f32r))

            # ---------------- weight prep (pipelined over row-tiles) -----
            # wiT[p_c, cc, e] = Wi_eff[e, 128cc+p]   (f32r)  Wi_eff = g/||v||*v
            # woT[p_e, ec, c] = Wo_eff[c, 128ec+p]   (bf16)  Wo_eff = 32*g/||v||*v
            wiT = wts.tile([P, C // P, E], f32r, tag="wiT")
            woT = wts.tile([P, E // P, C], bf16, tag="woT")
            g_i = wts.tile([P, E // P], f32, tag="g_i")
            g_o = wts.tile([P, C // P], f32, tag="g_o")
            nc.gpsimd.dma_start(out=g_i, in_=WIG.rearrange("(a p) -> p a", p=P))
            nc.gpsimd.dma_start(out=g_o, in_=WOG.rearrange("(a p) -> p a", p=P))

            for VSRC, G, WT, scale_extra in (
                (WIV, g_i, wiT, 1.0),
                (WOV, g_o, woT, CTX_SCALE),
            ):
                nrow = VSRC.shape[0] // P
                for rt in range(nrow):  # row-tile of the weight (dim0)
                    vsl = op.tile([P, 1024], f32, tag="stage")
                    nc.sync.dma_start(out=vsl, in_=VSRC[P * rt:P * (rt + 1), :])
                    sq = sqp.tile([P, 1024], f32, tag="sq")
                    ss = st.tile([P, 1], f32, tag="ss")
                    nc.scalar.activation(sq, vsl, ACT_F.Square, accum_out=ss)
                    srt = st.tile([P, 1], f32, tag="ss")
                    nc.scalar.activation(srt, ss, ACT_F.Sqrt)
                    rcs = st.tile([P, 1], f32, tag="ss")
                    nc.vector.reciprocal(rcs, srt)
                    fac = st.tile([P, 1], f32, tag="ss")
                    nc.vector.tensor_tensor(
                        out=fac, in0=rcs, in1=G[:, rt:rt + 1], op=ALU.mult)
                    if scale_extra != 1.0:
                        nc.vector.tensor_scalar_mul(fac, fac, float(scale_extra))
                    nc.vector.tensor_scalar_mul(vsl, vsl, fac)  # in place
                    for ft in range(1024 // P):  # transpose into WT
                        pt = ps.tile([P, P], f32, tag="ps")
                        nc.tensor.transpose(pt, vsl[:, P * ft:P * (ft + 1)], id_f)
                        nc.vector.tensor_copy(
                            WT[:, ft, P * rt:P * (rt + 1)], pt)

            # ---------------- pipelined chunk loop ----------------
            NTOT = NB * NCH
            state = {}

            def produce(ci):
                n, ch = divmod(ci, NCH)
                t0 = ch * TCH
                s = state[ci] = {}
                if ch == 0:
                    K_sb = kvp.tile([P, E // P, TS], f32r, tag="K")
                    nc.sync.dma_start(
                        out=K_sb,
                        in_=EK[n].rearrange("(ec p) s -> p ec s", p=P)
                        .bitcast(f32r))
                    V_bf = kvp.tile([P, TS // P, E], bf16, tag="V")
                    for sc in range(TS // P):
                        vst = op.tile([P, E], f32, tag="stage")
                        nc.sync.dma_start(
                            out=vst, in_=EV[n, P * sc:P * (sc + 1), :])
                        nc.vector.tensor_copy(V_bf[:, sc, :], vst)
                    state[("kv", n)] = (K_sb, V_bf)
                K_sb, V_bf = state[("kv", n)]

                x_t = xp.tile([P, NTT, C], f32r, tag="x")
                nc.sync.dma_start(
                    out=x_t,
                    in_=X[n, t0:t0 + TCH, :]
                    .rearrange("(tt p) c -> p tt c", p=P).bitcast(f32r))
                te_t = tep.tile([P, NTT, E], f32, tag="te")
                nc.sync.dma_start(
                    out=te_t,
                    in_=TE[n, t0:t0 + TCH, :]
                    .rearrange("(tt p) e -> p tt e", p=P))
                s["x"] = x_t

                # x^T tiles: [c-part, t-free]
                xT_t = xtp.tile([P, C // P, TCH], f32r, tag="xT")
                for cc in range(C // P):
                    for tt in range(NTT):
                        pt = ps.tile([P, P], f32r, tag="ps")
                        nc.tensor.matmul(
                            pt, lhsT=x_t[:, tt, P * cc:P * (cc + 1)],
                            rhs=id_r, is_transpose=True,
                            start=True, stop=True)
                        nc.vector.tensor_copy(
                            xT_t[:, cc, P * tt:P * (tt + 1)], pt)

                # mm1: h^T[e,t] (+te^T fused), evac Identity(scale, bias)
                hT_t = htp.tile([P, E // P, TCH], f32r, tag="hT")
                for et in range(E // P):
                    ph = ps.tile([P, TCH], f32, tag="ps")
                    for cc in range(C // P):
                        nc.tensor.matmul(
                            ph, lhsT=wiT[:, cc, P * et:P * (et + 1)],
                            rhs=xT_t[:, cc, :],
                            start=(cc == 0), stop=False)
                    for tt in range(NTT):
                        nc.tensor.matmul(
                            ph[:, P * tt:P * (tt + 1)],
                            lhsT=te_t[:, tt, P * et:P * (et + 1)],
                            rhs=id_f, is_transpose=True,
                            start=False, stop=(tt == NTT - 1))
                    nc.scalar.activation(
                        hT_t[:, et, :], ph, ACT_F.Identity,
                        scale=SQ5, bias=b_i_s[:, et:et + 1])

                # mm2 + softmax per t-tile
                attn_t = atp.tile([P, NTT, TS], f32, tag="attn")
                s["attn"] = attn_t
                for tt in range(NTT):
                    psc = []
                    for sh in range(2):
                        pb = ps.tile([P, 512], f32, tag="ps")
                        for ec in range(E // P):
                            nc.tensor.matmul(
                                pb,
                                lhsT=hT_t[:, ec, P * tt:P * (tt + 1)],
                                rhs=K_sb[:, ec, 512 * sh:512 * (sh + 1)],
                                start=(ec == 0), stop=(ec == 7))
                        psc.append(pb)
                    nm0 = st.tile([P, 1], f32, tag="sm")
                    nm1 = st.tile([P, 1], f32, tag="sm")
                    nc.vector.tensor_reduce(
                        nm0, psc[0], axis=AX.X, op=ALU.max, negate=True)
                    nc.vector.tensor_reduce(
                        nm1, psc[1], axis=AX.X, op=ALU.max, negate=True)
                    nm = st.tile([P, 1], f32, tag="sm")
                    nc.vector.tensor_tensor(
                        out=nm, in0=nm0, in1=nm1, op=ALU.min)
                    sm0 = st.tile([P, 1], f32, tag="sm")
                    sm1 = st.tile([P, 1], f32, tag="sm")
                    for sh, smx in ((0, sm0), (1, sm1)):
                        nc.scalar.activation(
                            attn_t[:, tt, 512 * sh:512 * (sh + 1)], psc[sh],
                            ACT_F.Exp, bias=nm, scale=1.0, accum_out=smx)
                    stot = st.tile([P, 1], f32, tag="sm")
                    nc.vector.tensor_tensor(
                        out=stot, in0=sm0, in1=sm1, op=ALU.add)
                    rcp = st.tile([P, 1], f32, tag="sm")
                    nc.vector.reciprocal(rcp, stot)
                    nc.vector.tensor_scalar_mul(
                        attn_t[:, tt, :], attn_t[:, tt, :], rcp)
                    nc.sync.dma_start(
                        out=ATTN[n, t0 + P * tt:t0 + P * (tt + 1), :],
                        in_=attn_t[:, tt, :])

            def consume(ci):
                n, ch = divmod(ci, NCH)
                t0 = ch * TCH
                s = state[ci]
                K_sb, V_bf = state[("kv", n)]
                attn_t = s["attn"]
                x_t = s["x"]

                attnT_t = attp.tile([P, TS // P, TCH], bf16, tag="attnT")
                for tt in range(NTT):
                    for sc in range(TS // P):
                        pt = ps.tile([P, P], f32, tag="ps")
                        nc.tensor.matmul(
                            pt, lhsT=attn_t[:, tt, P * sc:P * (sc + 1)],
                            rhs=id_f, is_transpose=True,
                            start=True, stop=True)
                        nc.vector.tensor_copy(
                            attnT_t[:, sc, P * tt:P * (tt + 1)], pt)

                # mm3: ctx^T[e,t] = V.T @ attn^T  (bf16)
                ctxT_t = ctp.tile([P, E // P, TCH], bf16, tag="ctxT")
                for et in range(E // P):
                    pc = ps.tile([P, TCH], f32, tag="ps")
                    for sc in range(TS // P):
                        nc.tensor.matmul(
                            pc, lhsT=V_bf[:, sc, P * et:P * (et + 1)],
                            rhs=attnT_t[:, sc, :],
                            start=(sc == 0), stop=(sc == 7))
                    nc.scalar.copy(ctxT_t[:, et, :], pc)

                # mm4: out[t,c] (+bias, +residual), evac * sqrt(.5)
                for tt in range(NTT):
                    out_t = op.tile([P, C], f32, tag="stage")
                    for cf in range(2):
                        po = ps.tile([P, 512], f32, tag="ps")
                        for ec in range(E // P):
                            nc.tensor.matmul(
                                po,
                                lhsT=ctxT_t[:, ec, P * tt:P * (tt + 1)],
                                rhs=woT[:, ec, 512 * cf:512 * (cf + 1)],
                                start=(ec == 0), stop=False)
                        nc.tensor.matmul(
                            po, lhsT=ones_r,
                            rhs=b_o_row[:, 512 * cf:512 * (cf + 1)],
                            start=False, stop=False)
                        nc.tensor.matmul(
                            po, lhsT=id_r,
                            rhs=x_t[:, tt, 512 * cf:512 * (cf + 1)],
                            start=False, stop=True)
                        nc.scalar.activation(
                            out_t[:, 512 * cf:512 * (cf + 1)], po,
                            ACT_F.Copy, scale=SQ5)
                    nc.sync.dma_start(
                        out=OUT[n, t0 + P * tt:t0 + P * (tt + 1), :],
                        in_=out_t)
                del s["attn"], s["x"]

            for ci in range(NTOT + 1):
                if ci < NTOT:
                    produce(ci)
                if ci > 0:
                    consume(ci - 1)

    nc.finalize()
    return nc


def _get_nc():
    global _CACHED
    if _CACHED is None:
        _CACHED = _build()
    return _CACHED


def kernel(x, target_embedding, encoder_keys, encoder_values,
           in_proj_v, in_proj_g, in_proj_b,
           out_proj_v, out_proj_g, out_proj_b,
           trace=False, **run_kwargs):
    from concourse.bass_utils import run_bass_kernel_spmd

    nc = _get_nc()
    f32 = np.float32
    x = np.ascontiguousarray(np.asarray(x, f32))
    te = np.ascontiguousarray(np.asarray(target_embedding, f32))
    ek = np.ascontiguousarray(np.asarray(encoder_keys, f32))
    ev = np.ascontiguousarray(np.asarray(encoder_values, f32))
    wiv = np.ascontiguousarray(np.asarray(in_proj_v, f32))
    wig = np.ascontiguousarray(np.asarray(in_proj_g, f32))
    wib = np.ascontiguousarray(np.asarray(in_proj_b, f32))
    wov = np.ascontiguousarray(np.asarray(out_proj_v, f32))
    wog = np.ascontiguousarray(np.asarray(out_proj_g, f32))
    wob = np.ascontiguousarray(np.asarray(out_proj_b, f32))

    in_maps = []
    for c in range(NCORES):
        sl = slice(NB * c, NB * (c + 1))
        in_maps.append({
            "x": x[sl], "te": te[sl], "ek": ek[sl], "ev": ev[sl],
            "wiv": wiv, "wig": wig, "wib": wib,
            "wov": wov, "wog": wog, "wob": wob,
        })
    res = run_bass_kernel_spmd(nc, in_maps, core_ids=list(range(NCORES)),
                               trace=trace, **run_kwargs)
    out = np.concatenate([r["out"] for r in res.results], axis=0)
    attn = np.concatenate([r["attn"] for r in res.results], axis=0)
    kernel.last_results = res
    return out, attn


# revision 12
# speedup vs baseline: 1.1830x; 1.0438x over previous
"""Trainium2 Bass kernel for nn_AttentionLayer (dense transformer block).

Per-batch computation (reference):
  Wi = wn(in_proj_v, in_proj_g)          # [E, C], rows unit-normed * g
  h  = (x @ Wi.T + b_i + te) * sqrt(.5)  # [Tq, E]
  scores = h @ K                         # [Tq, Ts]   (K = encoder_keys [E, Ts])
  attn = softmax(scores, -1)             # output #2
  ctx = (attn @ V) * 32                  # V = encoder_values [Ts, E]
  out = (ctx @ Wo.T + b_o + x) * sqrt(.5)

Sharding: data-parallel over batch N=16 across 8 cores (2 batches/core).

Per-core dataflow (everything on-device; fp32r for the precision-critical
h/scores path, bf16 for the attn@V / @Wo.T tail):
  mm1: h^T[e,t] = WiT.T @ x^T   -- lhsT = WiT (one-time PE-transposed,
       weight-norm factors folded per-partition), rhs = x^T (PE-transposed
       tiles), te^T fused as transpose-matmuls accumulated into the same
       PSUM group, bias via per-partition Identity-activation evac with
       scale=sqrt(.5).
  mm2: scores[t,s] = hT.T @ K   -- K natural layout, resident per batch.
  softmax over the free dim: DVE reduce_max(negate) -> ACT Exp(bias=-max,
       accum_out=rowsum) -> reciprocal -> normalize -> DMA attn out.
  mm3: ctx^T[e,t] = V.T @ attn^T -- V natural (bf16), attn^T PE-transposed.
  mm4: out[t,c] = ctxT.T @ WoT + b_o (K=1 matmul) + residual (identity
       matmul on x), evac with scale=sqrt(.5); *32 folded into WoT.

The chunk loop is software-pipelined: produce(ch) = {x/te load, x^T, mm1,
mm2, softmax}; consume(ch) = {attn^T, mm3, mm4, out}. consume(ch) is
emitted after produce(ch+1) so the softmax of chunk ch overlaps the
~18us of PE matmul work in produce(ch+1), keeping the PE gap-free (HAM
stays at K=8/8).
"""
import sys
sys.path.insert(0, "/opt/trn_rl_repo")
import math
import numpy as np

N, TQ, TS, C, E = 16, 1024, 1024, 1024, 1024
NCORES = 8
NB = N // NCORES          # batches per core
P = 128
TCH = 256                 # t-chunk (2 t-tiles)
NCH = TQ // TCH           # chunks per batch
NTT = TCH // P            # t-tiles per chunk
SQ5 = math.sqrt(0.5)
CTX_SCALE = TS * math.sqrt(1.0 / TS)   # 32.0

_CACHED = None


def _build():
    import concourse.mybir as mybir
    from concourse import bacc
    from concourse.tile import TileContext
    from concourse.masks import make_identity
    from contextlib import ExitStack

    f32 = mybir.dt.float32
    f32r = mybir.dt.float32r
    bf16 = mybir.dt.bfloat16
    AX = mybir.AxisListType
    ALU = mybir.AluOpType
    ACT_F = mybir.ActivationFunctionType

    nc = bacc.Bacc(trn_type="TRN2")
    X = nc.dram_tensor("x", [NB, TQ, C], f32, kind="ExternalInput")
    TE = nc.dram_tensor("te", [NB, TQ, E], f32, kind="ExternalInput")
    EK = nc.dram_tensor("ek", [NB, E, TS], f32, kind="ExternalInput")
    EV = nc.dram_tensor("ev", [NB, TS, E], f32, kind="ExternalInput")
    WIV = nc.dram_tensor("wiv", [E, C], f32, kind="ExternalInput")
    WIG = nc.dram_tensor("wig", [E], f32, kind="ExternalInput")
    WIB = nc.dram_tensor("wib", [E], f32, kind="ExternalInput")
    WOV = nc.dram_tensor("wov", [C, E], f32, kind="ExternalInput")
    WOG = nc.dram_tensor("wog", [C], f32, kind="ExternalInput")
    WOB = nc.dram_tensor("wob", [C], f32, kind="ExternalInput")
    OUT = nc.dram_tensor("out", [NB, TQ, C], f32, kind="ExternalOutput")
    ATTN = nc.dram_tensor("attn", [NB, TQ, TS], f32, kind="ExternalOutput")

    with TileContext(nc) as tc:
        ctx = ExitStack()
        with ctx:
            wts = ctx.enter_context(tc.tile_pool(name="wts", bufs=1))
            kvp = ctx.enter_context(tc.tile_pool(name="kvp", bufs=1))
            xp = ctx.enter_context(tc.tile_pool(name="xp", bufs=3))
            tep = ctx.enter_context(tc.tile_pool(name="tep", bufs=1))
            xtp = ctx.enter_context(tc.tile_pool(name="xtp", bufs=1))
            htp = ctx.enter_context(tc.tile_pool(name="htp", bufs=1))
            atp = ctx.enter_context(tc.tile_pool(name="atp", bufs=2))
            attp = ctx.enter_context(tc.tile_pool(name="attp", bufs=2))
            ctp = ctx.enter_context(tc.tile_pool(name="ctp", bufs=1))
            op = ctx.enter_context(tc.tile_pool(name="op", bufs=5))
            sqp = ctx.enter_context(tc.tile_pool(name="sqp", bufs=2))
            st = ctx.enter_context(tc.tile_pool(name="st", bufs=16))
            ps = ctx.enter_context(tc.tile_pool(name="ps", bufs=8, space="PSUM"))

            # ---------------- constants ----------------
            id_f = wts.tile([P, P], f32, tag="id_f")
            make_identity(nc, id_f)
            id_r = wts.tile([P, P], f32r, tag="id_r")
            nc.vector.tensor_copy(id_r, id_f.bitcast(f32r))
            ones_r = wts.tile([1, P], f32r, tag="ones")
            ones_f = wts.tile([1, P], f32, tag="ones_f")
            nc.vector.memset(ones_f, 1.0)
            nc.vector.tensor_copy(ones_r, ones_f.bitcast(f32r))

            # in-proj bias: per-partition [e] layout, pre-scaled by sqrt(.5)
            # (small loads go on the gpsimd queue to keep sync free for bulk)
            b_i_s = wts.tile([P, E // P], f32, tag="b_i")
            nc.gpsimd.dma_start(out=b_i_s, in_=WIB.rearrange("(a p) -> p a", p=P))
            nc.vector.tensor_scalar_mul(b_i_s, b_i_s, SQ5)
            # out-proj bias row (raw; sqrt(.5) applied at PSUM evac)
            b_o_row = wts.tile([1, C], f32r, tag="b_o")
            nc.gpsimd.dma_start(out=b_o_row, in_=WOB[None, :].bitcast(f32r))

            # ---------------- weight prep (pipelined over row-tiles) -----
            # wiT[p_c, cc, e] = Wi_eff[e, 128cc+p]   (f32r)  Wi_eff = g/||v||*v
            # woT[p_e, ec, c] = Wo_eff[c, 128ec+p]   (bf16)  Wo_eff = 32*g/||v||*v
            wiT = wts.tile([P, C // P, E], f32r, tag="wiT")
            woT = wts.tile([P, E // P, C], bf16, tag="woT")
            g_i = wts.tile([P, E // P], f32, tag="g_i")
            g_o = wts.tile([P, C // P], f32, tag="g_o")
            nc.gpsimd.dma_start(out=g_i, in_=WIG.rearrange("(a p) -> p a", p=P))
            nc.gpsimd.dma_start(out=g_o, in_=WOG.rearrange("(a p) -> p a", p=P))

            for VSRC, G, WT, scale_extra in (
                (WIV, g_i, wiT, 1.0),
                (WOV, g_o, woT, CTX_SCALE),
            ):
                nrow = VSRC.shape[0] // P
                for rt in range(nrow):  # row-tile of the weight (dim0)
                    vsl = op.tile([P, 1024], f32, tag="stage")
                    nc.sync.dma_start(out=vsl, in_=VSRC[P * rt:P * (rt + 1), :])
                    sq = sqp.tile([P, 1024], f32, tag="sq")
                    ss = st.tile([P, 1], f32, tag="ss")
                    nc.scalar.activation(sq, vsl, ACT_F.Square, accum_out=ss)
                    srt = st.tile([P, 1], f32, tag="ss")
                    nc.scalar.activation(srt, ss, ACT_F.Sqrt)
                    rcs = st.tile([P, 1], f32, tag="ss")
                    nc.vector.reciprocal(rcs, srt)
                    fac = st.tile([P, 1], f32, tag="ss")
                    nc.vector.tensor_tensor(
                        out=fac, in0=rcs, in1=G[:, rt:rt + 1], op=ALU.mult)
                    if scale_extra != 1.0:
                        nc.vector.tensor_scalar_mul(fac, fac, float(scale_extra))
                    nc.vector.tensor_scalar_mul(vsl, vsl, fac)  # in place
                    # transpose into WT: 4 tiles per PSUM bank, one wide evac
                    for half in range(2):
                        pt = ps.tile([P, 512], f32, tag="ps")
                        for j in range(4):
                            ft = 4 * half + j
                            nc.tensor.transpose(
                                pt[:, P * j:P * (j + 1)],
                                vsl[:, P * ft:P * (ft + 1)], id_f)
                        dst = WT[:, 4 * half:4 * half + 4, P * rt:P * (rt + 1)]
                        if half == 0:
                            nc.scalar.copy(dst, pt)
                        else:
                            nc.vector.tensor_copy(dst, pt)

            # ---------------- pipelined chunk loop ----------------
            NTOT = NB * NCH
            state = {}

            def produce(ci):
                n, ch = divmod(ci, NCH)
                t0 = ch * TCH
                s = state[ci] = {}
                if ch == 0:
                    K_sb = kvp.tile([P, E // P, TS], f32r, tag="K")
                    nc.sync.dma_start(
                        out=K_sb,
                        in_=EK[n].rearrange("(ec p) s -> p ec s", p=P)
                        .bitcast(f32r))
                    V_bf = kvp.tile([P, TS // P, E], bf16, tag="V")
                    for sc in range(TS // P):
                        vst = op.tile([P, E], f32, tag="stage")
                        nc.sync.dma_start(
                            out=vst, in_=EV[n, P * sc:P * (sc + 1), :])
                        nc.vector.tensor_copy(V_bf[:, sc, :], vst)
                    state[("kv", n)] = (K_sb, V_bf)
                K_sb, V_bf = state[("kv", n)]

                x_t = xp.tile([P, NTT, C], f32r, tag="x")
                nc.sync.dma_start(
                    out=x_t,
                    in_=X[n, t0:t0 + TCH, :]
                    .rearrange("(tt p) c -> p tt c", p=P).bitcast(f32r))
                te_t = tep.tile([P, NTT, E], f32, tag="te")
                nc.sync.dma_start(
                    out=te_t,
                    in_=TE[n, t0:t0 + TCH, :]
                    .rearrange("(tt p) e -> p tt e", p=P))
                s["x"] = x_t

                # x^T tiles: [c-part, t-free]
                xT_t = xtp.tile([P, C // P, TCH], f32r, tag="xT")
                for cc in range(C // P):
                    for tt in range(NTT):
                        pt = ps.tile([P, P], f32r, tag="ps")
                        nc.tensor.matmul(
                            pt, lhsT=x_t[:, tt, P * cc:P * (cc + 1)],
                            rhs=id_r, is_transpose=True,
                            start=True, stop=True)
                        nc.vector.tensor_copy(
                            xT_t[:, cc, P * tt:P * (tt + 1)], pt)

                # mm1: h^T[e,t] (+te^T fused), evac Identity(scale, bias)
                hT_t = htp.tile([P, E // P, TCH], f32r, tag="hT")
                for et in range(E // P):
                    ph = ps.tile([P, TCH], f32, tag="ps")
                    for cc in range(C // P):
                        nc.tensor.matmul(
                            ph, lhsT=wiT[:, cc, P * et:P * (et + 1)],
                            rhs=xT_t[:, cc, :],
                            start=(cc == 0), stop=False)
                    for tt in range(NTT):
                        nc.tensor.matmul(
                            ph[:, P * tt:P * (tt + 1)],
                            lhsT=te_t[:, tt, P * et:P * (et + 1)],
                            rhs=id_f, is_transpose=True,
                            start=False, stop=(tt == NTT - 1))
                    nc.scalar.activation(
                        hT_t[:, et, :], ph, ACT_F.Identity,
                        scale=SQ5, bias=b_i_s[:, et:et + 1])

                # mm2 + softmax per t-tile
                attn_t = atp.tile([P, NTT, TS], f32, tag="attn")
                s["attn"] = attn_t
                for tt in range(NTT):
                    psc = []
                    for sh in range(2):
                        pb = ps.tile([P, 512], f32, tag="ps")
                        for ec in range(E // P):
                            nc.tensor.matmul(
                                pb,
                                lhsT=hT_t[:, ec, P * tt:P * (tt + 1)],
                                rhs=K_sb[:, ec, 512 * sh:512 * (sh + 1)],
                                start=(ec == 0), stop=(ec == 7))
                        psc.append(pb)
                    nm0 = st.tile([P, 1], f32, tag="sm")
                    nm1 = st.tile([P, 1], f32, tag="sm")
                    nc.vector.tensor_reduce(
                        nm0, psc[0], axis=AX.X, op=ALU.max, negate=True)
                    nc.vector.tensor_reduce(
                        nm1, psc[1], axis=AX.X, op=ALU.max, negate=True)
                    nm = st.tile([P, 1], f32, tag="sm")
                    nc.vector.tensor_tensor(
                        out=nm, in0=nm0, in1=nm1, op=ALU.min)
                    sm0 = st.tile([P, 1], f32, tag="sm")
                    sm1 = st.tile([P, 1], f32, tag="sm")
                    for sh, smx in ((0, sm0), (1, sm1)):
                        nc.scalar.activation(
                            attn_t[:, tt, 512 * sh:512 * (sh + 1)], psc[sh],
                            ACT_F.Exp, bias=nm, scale=1.0, accum_out=smx)
                    stot = st.tile([P, 1], f32, tag="sm")
                    nc.vector.tensor_tensor(
                        out=stot, in0=sm0, in1=sm1, op=ALU.add)
                    rcp = st.tile([P, 1], f32, tag="sm")
                    nc.vector.reciprocal(rcp, stot)
                    nc.vector.tensor_scalar_mul(
                        attn_t[:, tt, :], attn_t[:, tt, :], rcp)
                    nc.sync.dma_start(
                        out=ATTN[n, t0 + P * tt:t0 + P * (tt + 1), :],
                        in_=attn_t[:, tt, :])

            def consume(ci):
                n, ch = divmod(ci, NCH)
                t0 = ch * TCH
                s = state[ci]
                K_sb, V_bf = state[("kv", n)]
                attn_t = s["attn"]
                x_t = s["x"]

                attnT_t = attp.tile([P, TS // P, TCH], bf16, tag="attnT")
                for tt in range(NTT):
                    for sc in range(TS // P):
                        pt = ps.tile([P, P], f32, tag="ps")
                        nc.tensor.matmul(
                            pt, lhsT=attn_t[:, tt, P * sc:P * (sc + 1)],
                            rhs=id_f, is_transpose=True,
                            start=True, stop=True)
                        nc.vector.tensor_copy(
                            attnT_t[:, sc, P * tt:P * (tt + 1)], pt)

                # mm3: ctx^T[e,t] = V.T @ attn^T  (bf16)
                ctxT_t = ctp.tile([P, E // P, TCH], bf16, tag="ctxT")
                for et in range(E // P):
                    pc = ps.tile([P, TCH], f32, tag="ps")
                    for sc in range(TS // P):
                        nc.tensor.matmul(
                            pc, lhsT=V_bf[:, sc, P * et:P * (et + 1)],
                            rhs=attnT_t[:, sc, :],
                            start=(sc == 0), stop=(sc == 7))
                    nc.scalar.copy(ctxT_t[:, et, :], pc)

                # mm4: out[t,c] (+bias, +residual), evac * sqrt(.5)
                for tt in range(NTT):
                    out_t = op.tile([P, C], f32, tag="stage")
                    for cf in range(2):
                        po = ps.tile([P, 512], f32, tag="ps")
                        for ec in range(E // P):
                            nc.tensor.matmul(
                                po,
                                lhsT=ctxT_t[:, ec, P * tt:P * (tt + 1)],
                                rhs=woT[:, ec, 512 * cf:512 * (cf + 1)],
                                start=(ec == 0), stop=False)
                        nc.tensor.matmul(
                            po, lhsT=ones_r,
                            rhs=b_o_row[:, 512 * cf:512 * (cf + 1)],
                            start=False, stop=False)
                        nc.tensor.matmul(
                            po, lhsT=id_r,
                            rhs=x_t[:, tt, 512 * cf:512 * (cf + 1)],
                            start=False, stop=True)
                        nc.scalar.activation(
                            out_t[:, 512 * cf:512 * (cf + 1)], po,
                            ACT_F.Copy, scale=SQ5)
                    nc.sync.dma_start(
                        out=OUT[n, t0 + P * tt:t0 + P * (tt + 1), :],
                        in_=out_t)
                del s["attn"], s["x"]

            for ci in range(NTOT + 1):
                if ci < NTOT:
                    produce(ci)
                if ci > 0:
                    consume(ci - 1)

    nc.finalize()
    return nc


def _get_nc():
    global _CACHED
    if _CACHED is None:
        _CACHED = _build()
    return _CACHED


def kernel(x, target_embedding, encoder_keys, encoder_values,
           in_proj_v, in_proj_g, in_proj_b,
           out_proj_v, out_proj_g, out_proj_b,
           trace=False, **run_kwargs):
    from concourse.bass_utils import run_bass_kernel_spmd

    nc = _get_nc()
    f32 = np.float32
    x = np.ascontiguousarray(np.asarray(x, f32))
    te = np.ascontiguousarray(np.asarray(target_embedding, f32))
    ek = np.ascontiguousarray(np.asarray(encoder_keys, f32))
    ev = np.ascontiguousarray(np.asarray(encoder_values, f32))
    wiv = np.ascontiguousarray(np.asarray(in_proj_v, f32))
    wig = np.ascontiguousarray(np.asarray(in_proj_g, f32))
    wib = np.ascontiguousarray(np.asarray(in_proj_b, f32))
    wov = np.ascontiguousarray(np.asarray(out_proj_v, f32))
    wog = np.ascontiguousarray(np.asarray(out_proj_g, f32))
    wob = np.ascontiguousarray(np.asarray(out_proj_b, f32))

    in_maps = []
    for c in range(NCORES):
        sl = slice(NB * c, NB * (c + 1))
        in_maps.append({
            "x": x[sl], "te": te[sl], "ek": ek[sl], "ev": ev[sl],
            "wiv": wiv, "wig": wig, "wib": wib,
            "wov": wov, "wog": wog, "wob": wob,
        })
    res = run_bass_kernel_spmd(nc, in_maps, core_ids=list(range(NCORES)),
                               trace=trace, **run_kwargs)
    out = np.concatenate([r["out"] for r in res.results], axis=0)
    attn = np.concatenate([r["attn"] for r in res.results], axis=0)
    kernel.last_results = res
    return out, attn
